# revision 43
# baseline (speedup 1.0000x reference)
"""GCN (2-layer, PyG GCNConv semantics) on 8 Trainium2 NeuronCores.

Strategy (dst-shard, graph-parallel), v2:
- Nodes sharded contiguously across 8 cores (12500 dsts/core).
- 3 SPMD dispatches:
    A: u1 = dinv * (x @ W1)            (x pre-transposed bf16, 4KB DMA runs)
    B: s1 = segsum(g1); agg1 = dinv*(s1+u1own)+b1; r1 = relu;
       v2 = dinv*r1; t2 = v2 @ W2      (outputs only t2, 0.4MB)
    C: s2 = segsum(g2); z = dinv*(s2+t2own)+b2; out = log_softmax(z)
- Segment-sum: edges packed 8-per-slot by destination; per 128-dst
  block, the first T_ID=4 slots of every dst go to "identity" chunks
  (slot partition == dst row, lhsT = static identity - no one-hot
  work), remaining slots to ~1 "overflow" chunk routed by an is_equal
  one-hot. All chunks of a block accumulate into one PSUM tile
  [128, 16f, 8sub]; one DVE reduce per block sums the 8 subslots.
  Chunk structure is common across cores (max-over-cores sizing) so a
  single SPMD program serves all 8 cores.
- The two per-edge value gathers (u1[src]/t2[src] for 3.2M edges) run
  on the host between dispatches (every on-device gather primitive in
  this toolchain was measured unusable: indirect DMA ~1.6us/row,
  GPSIMD gather ucode unloadable under this walrus build).
"""
import os
import sys
import numpy as np

sys.path.insert(0, "/opt/trn_rl_repo")

try:
    # NTFF profiling glue: the image's antenv lacks axon_hooks, which makes
    # run_bass_kernel_spmd(trace=True) crash. Provide it (and register the
    # ctypes hook) so tracing works when BASS_TRACE is set; harmless if not.
    import types as _types

    if "antenv.axon_hooks" not in sys.modules:
        _m = _types.ModuleType("antenv.axon_hooks")
        _st = {}
        _m.set_axon_ntff_profile_hook = lambda h: _st.__setitem__("h", h)
        _m.get_axon_ntff_profile_hook = lambda: _st.get("h")
        sys.modules["antenv.axon_hooks"] = _m
        from trn_agent_boot.trn_boot import _ntff_profile_via_ctypes

        _m.set_axon_ntff_profile_hook(
            _ntff_profile_via_ctypes("/opt/axon/libaxon_pjrt.so")
        )
except Exception:
    pass

import ml_dtypes
import concourse.bass as bass
import concourse.mybir as mybir
import concourse.tile as tile
from concourse.vector_clock import ScopedClock
import concourse.bass_utils as _bu
from concourse.bass_utils import run_bass_kernel_spmd

_orig_upload = _bu.upload_artifacts


def _safe_upload(tmpdir):
    try:
        return _orig_upload(tmpdir)
    except Exception:
        return "local://" + tmpdir


_bu.upload_artifacts = _safe_upload

BF16 = mybir.dt.bfloat16
F32 = mybir.dt.float32
AF = mybir.ActivationFunctionType
ALU = mybir.AluOpType
NPBF16 = ml_dtypes.bfloat16
NPF8 = ml_dtypes.float8_e4m3

G1_FP8 = True    # layer-1 gathered values in fp8 (e4m3)
G2_FP8 = True    # layer-2 gathered values dtype
X_FP8 = True     # x (dispatch A input) dtype

N_CORES = 8
PER_CORE = 12500
NT = 98              # 128-dst tiles per core (12544 padded)
PADDED = NT * 128
PACK = 8             # edges per slot (matmul N = 16 feats x PACK)
T_ID = 4             # identity chunks per block (slots 0..3 of each dst)
SC = 32              # chunks per g superchunk (DMA batch)
SCOV = 16            # overflow chunks per is_equal batch

# ---------------------------------------------------------------------------
# walrus workaround: only ONE sync-wait command per instruction is accepted.
# ---------------------------------------------------------------------------


def _patched_drain_and_barrier(self, tick_clock, wait_clock):
    nc = self.nc
    carrier = nc.sync.nop(nofuse=True, hint="drain_wait_carrier")
    wait_clock.add_sem_waits(carrier.ins, ScopedClock({None: tick_clock.global_clock}))
    si = carrier.ins.sync_info
    waits = list(si.on_wait or []) if si else []
    if len(waits) > 1:
        si.on_wait = waits[:1]
        for i in range(1, len(waits)):
            extra = nc.sync.nop(nofuse=True, hint="drain_wait_carrier")
            extra.ins.sync_info = mybir.SyncInfo(on_wait=waits[i : i + 1], on_update=[])
    nc.sync.drain()
    nc.all_engine_barrier()
    assert self.sems is not None
    popped = nc._tile_sem_poison_stack.pop()
    assert popped is self._sem_poison
    nc.clear_and_free_semaphores(list(self.sems.allocated().values()))
    nc.all_engine_barrier()


tile.TileContext._drain_and_barrier = _patched_drain_and_barrier


def _legalize_waits(nc, max_waits=1):
    n = [0]

    def mk_nop(engine, waits):
        n[0] += 1
        return mybir.InstNoOp(
            name=f"waitnop-{n[0]}",
            engine=engine,
            ins=[],
            outs=[],
            sync_info=mybir.SyncInfo(on_wait=list(waits), on_update=[]),
            text_hint="wait_carrier",
        )

    for f in nc.m.functions:
        for bb in f.blocks:
            out = []
            changed = False
            for inst in bb.instructions:
                si = inst.sync_info
                waits = list(si.on_wait or []) if si else []
                if len(waits) > max_waits:
                    changed = True
                    for i in range(0, len(waits) - max_waits, max_waits):
                        out.append(mk_nop(inst.engine, waits[i : i + max_waits]))
                    si.on_wait = waits[len(waits) - max_waits :]
                out.append(inst)
            if changed:
                bb.instructions = out
    return nc


# ---------------------------------------------------------------------------
# device kernel builders
# ---------------------------------------------------------------------------


def build_A(FC=4, xdt=BF16):
    """u1 = dinv * (x @ W1). xT host layout [128, NT, FC, 128]."""
    nc = bass.Bass()
    xT = nc.dram_tensor("xT", [128, NT, FC, 128], xdt, kind="ExternalInput")
    W1b = nc.dram_tensor("W1b", [128, FC, 16], BF16, kind="ExternalInput")
    dinva = nc.dram_tensor("dinva", [128, NT], F32, kind="ExternalInput")
    u1 = nc.dram_tensor("u1", [128, NT, 16], BF16, kind="ExternalOutput")
    TB = 8  # node-tiles per DMA batch (8KB per partition)
    with tile.TileContext(nc) as tc:
        with (
            tc.tile_pool(name="sbuf", bufs=3) as pool,
            tc.tile_pool(name="stat", bufs=1) as spool,
            tc.tile_pool(name="psum", bufs=8, space="PSUM") as pp,
        ):
            w1 = spool.tile([128, FC, 16], BF16)
            nc.sync.dma_start(out=w1[:], in_=W1b[:])
            da = spool.tile([128, NT], F32)
            nc.scalar.dma_start(out=da[:], in_=dinva[:])
            u1_sb = spool.tile([128, NT, 16], BF16)
            batches = [(0, 2), (2, 6)]
            while batches[-1][0] + batches[-1][1] < NT:
                s = batches[-1][0] + batches[-1][1]
                batches.append((s, min(TB, NT - s)))
            for bi, (t0, tb) in enumerate(batches):
                xt = pool.tile([128, TB, FC, 128], xdt, tag="xt")
                eng = (nc.sync, nc.scalar, nc.gpsimd)[bi % 3]
                eng.dma_start(out=xt[:, :tb], in_=xT[:, t0 : t0 + tb])
                for i in range(tb):
                    ps = pp.tile([128, 16], F32, tag="hps")
                    for fc in range(FC):
                        nc.tensor.matmul(
                            out=ps[:],
                            lhsT=xt[:, i, fc, :],
                            rhs=w1[:, fc, :],
                            start=(fc == 0),
                            stop=(fc == FC - 1),
                        )
                    t = t0 + i
                    nc.vector.tensor_tensor(
                        out=u1_sb[:, t, :],
                        in0=ps[:],
                        in1=da[:, t : t + 1].to_broadcast([128, 16]),
                        op=ALU.mult,
                    )
            nc.sync.dma_start(out=u1[:], in_=u1_sb[:])
    return _legalize_waits(nc)


def _emit_segsum(
    nc, pool, pp, g, oh, id_sb, blocks_nov, CH, CHOV, on_stripe,
    pre_hook=None, gdt=BF16,
):
    """Per-block psum scatter + subslot reduce, delivered in 4-block stripes.

    Per block: T_ID identity chunks (lhsT = id_sb) + blocks_nov[b] overflow
    chunks (lhsT = host-precomputed one-hot slices streamed from `oh`). All
    chunks of a block accumulate into one PSUM sub-tile; 4 blocks share a
    bank. After each stripe's DVE reduce, on_stripe(b0, nb, s4) consumes the
    [128, nb, 16] f32 stripe so the epilogue overlaps the remaining scatter.

    g and oh DMAs use staged schedules (small first batches so the PE starts
    fast); pre_hook() is emitted right after the first g DMA so secondary
    input loads queue behind it."""
    batches = [(0, 8), (8, 24)]
    while batches[-1][0] + batches[-1][1] < CH:
        s = batches[-1][0] + batches[-1][1]
        batches.append((s, min(SC, CH - s)))
    bi = 0
    batch_end = 0
    g_cur = None
    cur_start = 0
    oh_cur = None
    oh_start = 0
    oh_end = 0
    oi = 0
    P4 = None
    q = 0
    jov = 0
    NB = len(blocks_nov)
    for b, nov in enumerate(blocks_nov):
        if b % 4 == 0:
            P4 = pp.tile([128, 4, 16, PACK], F32, tag="pblk")
        nch = T_ID + nov
        for k in range(nch):
            if q == batch_end:
                cur_start, w = batches[bi]
                g_cur = pool.tile([128, SC, 16, PACK], gdt, tag="gsc")
                eng = (nc.sync, nc.scalar, nc.gpsimd)[bi % 3]
                eng.dma_start(out=g_cur[:, :w], in_=g[:, cur_start : cur_start + w])
                batch_end = cur_start + w
                bi += 1
                if pre_hook is not None:
                    pre_hook()
                    pre_hook = None
            if k >= T_ID:
                if jov == oh_end:
                    oh_start = jov
                    wov = min(4 if oi == 0 else SCOV, CHOV - jov)
                    oh_cur = pool.tile([128, SCOV, 128], gdt, tag="ohb")
                    eng = (nc.gpsimd, nc.sync, nc.scalar)[oi % 3]
                    eng.dma_start(
                        out=oh_cur[:, :wov], in_=oh[:, oh_start : oh_start + wov]
                    )
                    oh_end = oh_start + wov
                    oi += 1
                lhsT = oh_cur[:, jov - oh_start, :]
                jov += 1
            else:
                lhsT = id_sb[:]
            nc.tensor.matmul(
                out=P4[:, b % 4],
                lhsT=lhsT,
                rhs=g_cur[:, q - cur_start],
                start=(k == 0),
                stop=(k == nch - 1),
            )
            q += 1
        if b % 4 == 3 or b == NB - 1:
            b0 = (b // 4) * 4
            nb = b - b0 + 1
            s4 = pool.tile([128, 4, 16], F32, tag="s4")
            nc.vector.tensor_reduce(
                out=s4[:, :nb],
                in_=P4[:, :nb],
                axis=mybir.AxisListType.X,
                op=ALU.add,
            )
            on_stripe(b0, nb, s4)


def build_B(CH, CHOV, blocks_nov, gdt=BF16):
    """s1 -> agg1 -> relu -> v2 -> t2 = v2 @ W2 (sole output), striped."""
    nc = bass.Bass()
    g = nc.dram_tensor("g", [128, CH, 16, PACK], gdt, kind="ExternalInput")
    oh = nc.dram_tensor("oh", [128, CHOV, 128], gdt, kind="ExternalInput")
    u1own = nc.dram_tensor("u1own", [128, NT, 16], BF16, kind="ExternalInput")
    dinva = nc.dram_tensor("dinva", [128, NT], F32, kind="ExternalInput")
    W2q = nc.dram_tensor("W2q", [64, 4, 16], BF16, kind="ExternalInput")
    identT = nc.dram_tensor("identT", [128, 128], BF16, kind="ExternalInput")
    t2 = nc.dram_tensor("t2", [128, NT, 16], BF16, kind="ExternalOutput")
    with tile.TileContext(nc) as tc:
        with (
            tc.tile_pool(name="sbuf", bufs=3) as pool,
            tc.tile_pool(name="stat", bufs=1) as spool,
            tc.tile_pool(name="psum", bufs=6, space="PSUM") as pp,
            tc.tile_pool(name="psumt", bufs=1, space="PSUM") as ppt,
        ):
            id_sb = spool.tile([128, 128], BF16)
            nc.sync.dma_start(out=id_sb[:], in_=identT[:])
            u1o_bf = spool.tile([128, NT, 16], BF16)
            da = spool.tile([128, NT], F32)
            w2q_sb = spool.tile([64, 4, 16], BF16)
            u1o = spool.tile([128, NT, 16], F32)
            t2_sb = spool.tile([128, NT, 16], BF16)

            def pre_hook():
                nc.scalar.dma_start(out=u1o_bf[:], in_=u1own[:])
                nc.scalar.dma_start(out=da[:], in_=dinva[:])
                nc.scalar.dma_start(out=w2q_sb[:], in_=W2q[:])
                nc.scalar.copy(out=u1o[:], in_=u1o_bf[:])

            def on_stripe(b0, nb, s4):
                sl = slice(b0, b0 + nb)
                agg = pool.tile([128, 4, 16], F32, tag="agg")
                nc.gpsimd.tensor_tensor(
                    out=agg[:, :nb], in0=s4[:, :nb], in1=u1o[:, sl], op=ALU.add
                )
                nc.gpsimd.tensor_tensor(
                    out=agg[:, :nb], in0=agg[:, :nb],
                    in1=da[:, sl].to_broadcast([128, nb, 16]), op=ALU.mult,
                )
                r4 = pool.tile([128, 4, 16], F32, tag="r4")
                nc.scalar.activation(out=r4[:, :nb], in_=agg[:, :nb], func=AF.Relu)
                v4 = pool.tile([128, 4, 16], BF16, tag="v4")
                nc.vector.tensor_tensor(
                    out=v4[:, :nb], in0=r4[:, :nb],
                    in1=da[:, sl].to_broadcast([128, nb, 16]), op=ALU.mult,
                )
                if nb < 4:
                    nc.vector.memset(v4[:, nb:, :], 0.0)
                trps = ppt.tile([64, 128], BF16, tag="trps")
                nc.tensor.transpose(out=trps[:], in_=v4[:], identity=id_sb[:])
                v2T = pool.tile([64, 128], BF16, tag="v2T")
                nc.scalar.copy(out=v2T[:], in_=trps[:])
                z4 = ppt.tile([128, 4, 16], F32, tag="z4")
                for j in range(nb):
                    nc.tensor.matmul(
                        out=z4[:, j], lhsT=v2T[:], rhs=w2q_sb[:, j, :],
                        start=True, stop=True,
                    )
                nc.scalar.copy(out=t2_sb[:, sl, :], in_=z4[:, :nb])
                nc.sync.dma_start(out=t2[:, sl, :], in_=t2_sb[:, sl, :])

            _emit_segsum(
                nc, pool, pp, g, oh, id_sb, blocks_nov, CH, CHOV,
                on_stripe, pre_hook=pre_hook, gdt=gdt,
            )
    return _legalize_waits(nc)


def build_C(CH, CHOV, blocks_nov, gdt=BF16):
    """s2 -> z = dinv*(s2 + t2own) + b2 -> log_softmax, striped."""
    nc = bass.Bass()
    g = nc.dram_tensor("g", [128, CH, 16, PACK], gdt, kind="ExternalInput")
    oh = nc.dram_tensor("oh", [128, CHOV, 128], gdt, kind="ExternalInput")
    t2own = nc.dram_tensor("t2own", [128, NT, 16], BF16, kind="ExternalInput")
    dinva = nc.dram_tensor("dinva", [128, NT], F32, kind="ExternalInput")
    identT = nc.dram_tensor("identT", [128, 128], BF16, kind="ExternalInput")
    outd = nc.dram_tensor("outd", [128, NT, 16], F32, kind="ExternalOutput")
    with tile.TileContext(nc) as tc:
        with (
            tc.tile_pool(name="sbuf", bufs=3) as pool,
            tc.tile_pool(name="stat", bufs=1) as spool,
            tc.tile_pool(name="psum", bufs=8, space="PSUM") as pp,
        ):
            id_sb = spool.tile([128, 128], BF16)
            nc.sync.dma_start(out=id_sb[:], in_=identT[:])
            t2o_bf = spool.tile([128, NT, 16], BF16)
            da = spool.tile([128, NT], F32)
            t2o = spool.tile([128, NT, 16], F32)
            o_sb = spool.tile([128, NT, 16], F32)

            def pre_hook():
                nc.scalar.dma_start(out=t2o_bf[:], in_=t2own[:])
                nc.scalar.dma_start(out=da[:], in_=dinva[:])
                nc.scalar.copy(out=t2o[:], in_=t2o_bf[:])

            def on_stripe(b0, nb, s4):
                sl = slice(b0, b0 + nb)
                z = pool.tile([128, 4, 16], F32, tag="zs")
                nc.gpsimd.tensor_tensor(
                    out=z[:, :nb], in0=s4[:, :nb], in1=t2o[:, sl], op=ALU.add
                )
                nc.gpsimd.tensor_tensor(
                    out=z[:, :nb], in0=z[:, :nb],
                    in1=da[:, sl].to_broadcast([128, nb, 16]), op=ALU.mult,
                )
                m4 = pool.tile([128, 4], F32, tag="m4")
                nc.vector.tensor_reduce(
                    out=m4[:, :nb], in_=z[:, :nb], axis=mybir.AxisListType.X,
                    op=ALU.max,
                )
                zc = pool.tile([128, 4, 16], F32, tag="zc")
                nc.vector.tensor_tensor(
                    out=zc[:, :nb], in0=z[:, :nb],
                    in1=m4[:, :nb].to_broadcast([128, nb, 16]), op=ALU.subtract,
                )
                e4 = pool.tile([128, 4, 16], F32, tag="e4")
                nc.scalar.activation(out=e4[:, :nb], in_=zc[:, :nb], func=AF.Exp)
                ss = pool.tile([128, 4], F32, tag="ss")
                nc.vector.tensor_reduce(
                    out=ss[:, :nb], in_=e4[:, :nb], axis=mybir.AxisListType.X,
                    op=ALU.add,
                )
                lse = pool.tile([128, 4], F32, tag="lse")
                nc.scalar.activation(out=lse[:, :nb], in_=ss[:, :nb], func=AF.Ln)
                nc.vector.tensor_tensor(
                    out=o_sb[:, sl, :], in0=zc[:, :nb],
                    in1=lse[:, :nb].to_broadcast([128, nb, 16]), op=ALU.subtract,
                )
                nc.sync.dma_start(out=outd[:, sl, :], in_=o_sb[:, sl, :])

            _emit_segsum(
                nc, pool, pp, g, oh, id_sb, blocks_nov, CH, CHOV,
                on_stripe, pre_hook=pre_hook, gdt=gdt,
            )
    return _legalize_waits(nc)


# ---------------------------------------------------------------------------
# host side
# ---------------------------------------------------------------------------


def _preprocess(edge_index, n_nodes):
    """Sort edges by dst; build the common chunk structure (T_ID identity +
    n_ov overflow chunks per 128-dst block) + per-core slot metadata."""
    src = np.asarray(edge_index[0])
    dst = np.asarray(edge_index[1])
    deg = np.bincount(dst, minlength=n_nodes).astype(np.float32) + 1.0
    dinv = (1.0 / np.sqrt(deg)).astype(np.float32)

    order = np.argsort(dst, kind="stable")
    sdst = dst[order]
    ssrc = src[order]
    bounds = np.searchsorted(sdst, np.arange(N_CORES + 1) * PER_CORE)

    # per-core local in-degree and slot counts
    deg_loc = np.zeros((N_CORES, PADDED), np.int64)
    core_edges = []
    for c in range(N_CORES):
        lo, hi = bounds[c], bounds[c + 1]
        ld = sdst[lo:hi] - c * PER_CORE
        deg_loc[c, : PER_CORE] = np.bincount(ld, minlength=PER_CORE)
        core_edges.append((ld, ssrc[lo:hi]))
    nslots = -(-deg_loc // PACK)                 # [8, PADDED] ceil div
    ovslots = np.maximum(nslots - T_ID, 0)       # [8, PADDED]

    # common structure: overflow chunk count per block = max over cores
    ov_per_block = ovslots.reshape(N_CORES, NT, 128).sum(axis=2)  # [8, NT]
    n_ov = -(-ov_per_block.max(axis=0) // 128)   # [NT]
    blocks_nov = tuple(int(v) for v in n_ov)
    chunk_base = np.concatenate([[0], np.cumsum(T_ID + n_ov)])    # [NT+1]
    CH = int(chunk_base[-1])
    ov_idx_base = np.concatenate([[0], np.cumsum(n_ov)])          # [NT+1]
    CHOV = max(int(ov_idx_base[-1]), 1)

    sent = N_CORES * PADDED  # sentinel row (zeros) in gather tables
    oh_arrs, sidx_arrs = [], []
    blk_of_dst = np.arange(PADDED) >> 7
    for c in range(N_CORES):
        ov = ovslots[c]
        # exclusive cumsum of overflow slots within each block
        ovc = np.cumsum(ov) - ov
        blk_start = blk_of_dst << 7
        ovbase = ovc - ovc[blk_start]            # [PADDED]
        ld, esrc = core_edges[c]
        gstart = np.concatenate([[0], np.cumsum(deg_loc[c])])
        rank = np.arange(len(ld)) - gstart[ld]
        k_e = rank // PACK
        c_e = rank % PACK
        blk = ld >> 7
        is_id = k_e < T_ID
        q_id = chunk_base[blk] + k_e
        p_id = ld & 127
        ovpos = ovbase[ld] + (k_e - T_ID)
        q_ov = chunk_base[blk] + T_ID + ovpos // 128
        p_ov = ovpos % 128
        q_e = np.where(is_id, q_id, q_ov)
        p_e = np.where(is_id, p_id, p_ov)
        # gather row index: src node -> (core, p, t) -> core*PADDED + p*NT + t
        sc_, rr = esrc // PER_CORE, esrc % PER_CORE
        grow = sc_ * PADDED + (rr % 128) * NT + rr // 128
        sidx = np.full((128, CH, PACK), sent, np.int64)
        sidx[p_e, q_e, c_e] = grow
        # precomputed overflow one-hots [128 slot, CHOV, 128 row]
        oh = np.zeros((128, CHOV, 128), np.uint8)
        m = (~is_id) & (c_e == 0)
        qovc = ov_idx_base[blk[m]] + ovpos[m] // 128
        oh[p_ov[m], qovc, ld[m] & 127] = 1
        oh_arrs.append(oh)
        sidx_arrs.append(sidx)
    return dinv, CH, CHOV, blocks_nov, oh_arrs, sidx_arrs


_CACHE = {}
LAST_HW_NS = None
LAST_TIMES = {}


def _record(tag, res, t_wall):
    global LAST_HW_NS
    LAST_TIMES[tag] = t_wall
    if res.exec_time_ns is not None:
        LAST_HW_NS = (LAST_HW_NS or 0) + res.exec_time_ns


def _gather_g(table, sidx):
    """table [8*PADDED+1, 16] bf16, sidx [128, CH, PACK] -> [128, CH, 16, PACK]."""
    vals = table[sidx]  # [128, CH, PACK, 16]
    return np.ascontiguousarray(vals.transpose(0, 1, 3, 2))


def kernel(x, W1, b1, W2, b2, edge_index):
    global LAST_HW_NS
    LAST_HW_NS = None
    LAST_TIMES.clear()
    import time as _time

    x = np.asarray(x, dtype=np.float32)
    W1 = np.asarray(W1, dtype=np.float32)
    b1 = np.asarray(b1, dtype=np.float32)
    W2 = np.asarray(W2, dtype=np.float32)
    b2 = np.asarray(b2, dtype=np.float32)
    edge_index = np.asarray(edge_index)
    n_nodes, fin = x.shape
    FC = fin // 128

    t0 = _time.time()
    dinv, CH, CHOV, blocks_nov, oh_arrs, sidx_arrs = _preprocess(
        edge_index, n_nodes
    )
    LAST_TIMES["preprocess"] = _time.time() - t0

    key = (n_nodes, CH, CHOV, blocks_nov, G1_FP8, G2_FP8, X_FP8)
    if key not in _CACHE:
        F8 = mybir.dt.float8e4
        _CACHE[key] = (
            build_A(FC, xdt=F8 if X_FP8 else BF16),
            build_B(CH, CHOV, blocks_nov, gdt=F8 if G1_FP8 else BF16),
            build_C(CH, CHOV, blocks_nov, gdt=F8 if G2_FP8 else BF16),
        )
    ncA, ncB, ncC = _CACHE[key]
    cores = list(range(N_CORES))

    # ---- static per-core arrays ----
    t0 = _time.time()
    W1r = np.ascontiguousarray(
        W1.astype(NPBF16).reshape(FC, 128, 16).transpose(1, 0, 2)
    )
    dinva_c = []
    for c in cores:
        dv = np.ones(PADDED, np.float32)
        dv[:PER_CORE] = dinv[c * PER_CORE : (c + 1) * PER_CORE]
        dinva_c.append(np.ascontiguousarray(dv.reshape(NT, 128).T))
    oh1_c = [a.astype(NPF8 if G1_FP8 else NPBF16) for a in oh_arrs]
    oh2_c = (
        oh1_c if G1_FP8 == G2_FP8
        else [a.astype(NPF8 if G2_FP8 else NPBF16) for a in oh_arrs]
    )
    W2bf = W2.astype(NPBF16)
    rdeg_c = []  # sqrt(deg) per core in [128, NT] layout (1/dinva)
    for c in cores:
        rdeg_c.append((1.0 / dinva_c[c]).astype(np.float32))
    w2q = np.zeros((64, 4, 16), NPBF16)
    for j in range(4):
        w2q[16 * j : 16 * j + 16, j] = W2bf
    ident_np = np.eye(128, dtype=np.float32).astype(NPBF16)

    # ---- dispatch A ----
    in_A = []
    xnp = NPF8 if X_FP8 else NPBF16
    for c in cores:
        xs = x[c * PER_CORE : (c + 1) * PER_CORE]
        xp = np.zeros((PADDED, fin), xnp)
        xp[: xs.shape[0]] = xs.astype(xnp)
        xTr = np.ascontiguousarray(
            xp.reshape(NT, 128, FC, 128).transpose(3, 0, 2, 1)
        )  # [128 f_lo, NT, FC, 128 n]
        in_A.append({"xT": xTr, "W1b": W1r, "dinva": dinva_c[c]})
    LAST_TIMES["prepA"] = _time.time() - t0
    t0 = _time.time()
    resA = run_bass_kernel_spmd(ncA, in_A, core_ids=cores)
    _record("dispatchA", resA, _time.time() - t0)
    u1s = [resA.results[c]["u1"] for c in cores]  # [128, NT, 16] bf16

    # ---- host gather for layer 1 ----
    t0 = _time.time()
    table1 = np.concatenate(
        [u1s[c].reshape(PADDED, 16) for c in cores] + [np.zeros((1, 16), NPBF16)],
        axis=0,
    )
    if G1_FP8:
        table1 = table1.astype(NPF8)
    in_B = []
    for c in cores:
        # fold the post-norm bias: dinv*(s + u1own + b1*sqrt(deg)) == dinv*(s+u1own) + b1
        u1f = u1s[c].astype(np.float32) + b1[None, None, :] * rdeg_c[c][:, :, None]
        in_B.append(
            {
                "g": _gather_g(table1, sidx_arrs[c]),
                "oh": oh1_c[c],
                "u1own": u1f.astype(NPBF16),
                "dinva": dinva_c[c],
                "W2q": w2q,
                "identT": ident_np,
            }
        )
    LAST_TIMES["gather1"] = _time.time() - t0
    t0 = _time.time()
    resB = run_bass_kernel_spmd(ncB, in_B, core_ids=cores)
    _record("dispatchB", resB, _time.time() - t0)
    t2s = [resB.results[c]["t2"] for c in cores]

    # ---- host gather for layer 2 ----
    t0 = _time.time()
    table2 = np.concatenate(
        [t2s[c].reshape(PADDED, 16) for c in cores] + [np.zeros((1, 16), NPBF16)],
        axis=0,
    )
    if G2_FP8:
        table2 = table2.astype(NPF8)
    in_C = []
    for c in cores:
        t2f = t2s[c].astype(np.float32) + b2[None, None, :] * rdeg_c[c][:, :, None]
        in_C.append(
            {
                "g": _gather_g(table2, sidx_arrs[c]),
                "oh": oh2_c[c],
                "t2own": t2f.astype(NPBF16),
                "dinva": dinva_c[c],
                "identT": ident_np,
            }
        )
    LAST_TIMES["gather2"] = _time.time() - t0
    t0 = _time.time()
    resC = run_bass_kernel_spmd(ncC, in_C, core_ids=cores)
    _record("dispatchC", resC, _time.time() - t0)
    out = np.concatenate(
        [
            resC.results[c]["outd"].transpose(1, 0, 2).reshape(PADDED, 16)[:PER_CORE]
            for c in cores
        ],
        axis=0,
    ).astype(np.float32)
    return out


# revision 44
# speedup vs baseline: 1.0301x; 1.0301x over previous
"""GCN (2-layer, PyG GCNConv semantics) on 8 Trainium2 NeuronCores.

Strategy (dst-shard, graph-parallel), v2:
- Nodes sharded contiguously across 8 cores (12500 dsts/core).
- 3 SPMD dispatches:
    A: u1 = dinv * (x @ W1)            (x pre-transposed bf16, 4KB DMA runs)
    B: s1 = segsum(g1); agg1 = dinv*(s1+u1own)+b1; r1 = relu;
       v2 = dinv*r1; t2 = v2 @ W2      (outputs only t2, 0.4MB)
    C: s2 = segsum(g2); z = dinv*(s2+t2own)+b2; out = log_softmax(z)
- Segment-sum: edges packed 8-per-slot by destination; per 128-dst
  block, the first T_ID=4 slots of every dst go to "identity" chunks
  (slot partition == dst row, lhsT = static identity - no one-hot
  work), remaining slots to ~1 "overflow" chunk routed by an is_equal
  one-hot. All chunks of a block accumulate into one PSUM tile
  [128, 16f, 8sub]; one DVE reduce per block sums the 8 subslots.
  Chunk structure is common across cores (max-over-cores sizing) so a
  single SPMD program serves all 8 cores.
- The two per-edge value gathers (u1[src]/t2[src] for 3.2M edges) run
  on the host between dispatches (every on-device gather primitive in
  this toolchain was measured unusable: indirect DMA ~1.6us/row,
  GPSIMD gather ucode unloadable under this walrus build).
"""
import os
import sys
import numpy as np

sys.path.insert(0, "/opt/trn_rl_repo")

try:
    # NTFF profiling glue: the image's antenv lacks axon_hooks, which makes
    # run_bass_kernel_spmd(trace=True) crash. Provide it (and register the
    # ctypes hook) so tracing works when BASS_TRACE is set; harmless if not.
    import types as _types

    if "antenv.axon_hooks" not in sys.modules:
        _m = _types.ModuleType("antenv.axon_hooks")
        _st = {}
        _m.set_axon_ntff_profile_hook = lambda h: _st.__setitem__("h", h)
        _m.get_axon_ntff_profile_hook = lambda: _st.get("h")
        sys.modules["antenv.axon_hooks"] = _m
        from trn_agent_boot.trn_boot import _ntff_profile_via_ctypes

        _m.set_axon_ntff_profile_hook(
            _ntff_profile_via_ctypes("/opt/axon/libaxon_pjrt.so")
        )
except Exception:
    pass

import ml_dtypes
import concourse.bass as bass
import concourse.mybir as mybir
import concourse.tile as tile
from concourse.vector_clock import ScopedClock
import concourse.bass_utils as _bu
from concourse.bass_utils import run_bass_kernel_spmd

_orig_upload = _bu.upload_artifacts


def _safe_upload(tmpdir):
    try:
        return _orig_upload(tmpdir)
    except Exception:
        return "local://" + tmpdir


_bu.upload_artifacts = _safe_upload

BF16 = mybir.dt.bfloat16
F32 = mybir.dt.float32
AF = mybir.ActivationFunctionType
ALU = mybir.AluOpType
NPBF16 = ml_dtypes.bfloat16
NPF8 = ml_dtypes.float8_e4m3

G1_FP8 = True    # layer-1 gathered values in fp8 (e4m3)
G2_FP8 = True    # layer-2 gathered values dtype
X_FP8 = True     # x (dispatch A input) dtype

N_CORES = 8
PER_CORE = 12500
NT = 98              # 128-dst tiles per core (12544 padded)
PADDED = NT * 128
PACK = 8             # edges per slot (matmul N = 16 feats x PACK)
T_ID = 4             # identity chunks per block (slots 0..3 of each dst)
SC = 32              # chunks per g superchunk (DMA batch)
SCOV = 16            # overflow chunks per is_equal batch

# ---------------------------------------------------------------------------
# walrus workaround: only ONE sync-wait command per instruction is accepted.
# ---------------------------------------------------------------------------


def _patched_drain_and_barrier(self, tick_clock, wait_clock):
    nc = self.nc
    carrier = nc.sync.nop(nofuse=True, hint="drain_wait_carrier")
    wait_clock.add_sem_waits(carrier.ins, ScopedClock({None: tick_clock.global_clock}))
    si = carrier.ins.sync_info
    waits = list(si.on_wait or []) if si else []
    if len(waits) > 1:
        si.on_wait = waits[:1]
        for i in range(1, len(waits)):
            extra = nc.sync.nop(nofuse=True, hint="drain_wait_carrier")
            extra.ins.sync_info = mybir.SyncInfo(on_wait=waits[i : i + 1], on_update=[])
    nc.sync.drain()
    nc.all_engine_barrier()
    assert self.sems is not None
    popped = nc._tile_sem_poison_stack.pop()
    assert popped is self._sem_poison
    nc.clear_and_free_semaphores(list(self.sems.allocated().values()))
    nc.all_engine_barrier()


tile.TileContext._drain_and_barrier = _patched_drain_and_barrier


def _legalize_waits(nc, max_waits=1):
    n = [0]

    def mk_nop(engine, waits):
        n[0] += 1
        return mybir.InstNoOp(
            name=f"waitnop-{n[0]}",
            engine=engine,
            ins=[],
            outs=[],
            sync_info=mybir.SyncInfo(on_wait=list(waits), on_update=[]),
            text_hint="wait_carrier",
        )

    for f in nc.m.functions:
        for bb in f.blocks:
            out = []
            changed = False
            for inst in bb.instructions:
                si = inst.sync_info
                waits = list(si.on_wait or []) if si else []
                if len(waits) > max_waits:
                    changed = True
                    for i in range(0, len(waits) - max_waits, max_waits):
                        out.append(mk_nop(inst.engine, waits[i : i + max_waits]))
                    si.on_wait = waits[len(waits) - max_waits :]
                out.append(inst)
            if changed:
                bb.instructions = out
    return nc


# ---------------------------------------------------------------------------
# device kernel builders
# ---------------------------------------------------------------------------


def build_A(FC=4, xdt=BF16):
    """u1 = dinv * (x @ W1). xT host layout [128, NT, FC, 128]."""
    nc = bass.Bass()
    xT = nc.dram_tensor("xT", [128, NT, FC, 128], xdt, kind="ExternalInput")
    W1b = nc.dram_tensor("W1b", [128, FC, 16], BF16, kind="ExternalInput")
    dinva = nc.dram_tensor("dinva", [128, NT], F32, kind="ExternalInput")
    u1 = nc.dram_tensor("u1", [128, NT, 16], BF16, kind="ExternalOutput")
    TB = 8  # node-tiles per DMA batch (8KB per partition)
    with tile.TileContext(nc) as tc:
        with (
            tc.tile_pool(name="sbuf", bufs=3) as pool,
            tc.tile_pool(name="stat", bufs=1) as spool,
            tc.tile_pool(name="psum", bufs=8, space="PSUM") as pp,
        ):
            w1 = spool.tile([128, FC, 16], BF16)
            nc.sync.dma_start(out=w1[:], in_=W1b[:])
            da = spool.tile([128, NT], F32)
            nc.scalar.dma_start(out=da[:], in_=dinva[:])
            u1_sb = spool.tile([128, NT, 16], BF16)
            batches = [(0, 2), (2, 6)]
            while batches[-1][0] + batches[-1][1] < NT:
                s = batches[-1][0] + batches[-1][1]
                batches.append((s, min(TB, NT - s)))
            for bi, (t0, tb) in enumerate(batches):
                xt = pool.tile([128, TB, FC, 128], xdt, tag="xt")
                eng = nc.sync if bi % 2 == 0 else nc.scalar
                eng.dma_start(out=xt[:, :tb], in_=xT[:, t0 : t0 + tb])
                for i in range(tb):
                    ps = pp.tile([128, 16], F32, tag="hps")
                    for fc in range(FC):
                        nc.tensor.matmul(
                            out=ps[:],
                            lhsT=xt[:, i, fc, :],
                            rhs=w1[:, fc, :],
                            start=(fc == 0),
                            stop=(fc == FC - 1),
                        )
                    t = t0 + i
                    nc.vector.tensor_tensor(
                        out=u1_sb[:, t, :],
                        in0=ps[:],
                        in1=da[:, t : t + 1].to_broadcast([128, 16]),
                        op=ALU.mult,
                    )
            nc.sync.dma_start(out=u1[:], in_=u1_sb[:])
    return _legalize_waits(nc)


def _emit_segsum(
    nc, pool, pp, g, oh, id_sb, blocks_nov, CH, CHOV, on_stripe,
    pre_hook=None, gdt=BF16,
):
    """Per-block psum scatter + subslot reduce, delivered in 4-block stripes.

    Per block: T_ID identity chunks (lhsT = id_sb) + blocks_nov[b] overflow
    chunks (lhsT = host-precomputed one-hot slices streamed from `oh`). All
    chunks of a block accumulate into one PSUM sub-tile; 4 blocks share a
    bank. After each stripe's DVE reduce, on_stripe(b0, nb, s4) consumes the
    [128, nb, 16] f32 stripe so the epilogue overlaps the remaining scatter.

    g and oh DMAs use staged schedules (small first batches so the PE starts
    fast); pre_hook() is emitted right after the first g DMA so secondary
    input loads queue behind it."""
    batches = [(0, 8), (8, 24)]
    while batches[-1][0] + batches[-1][1] < CH:
        s = batches[-1][0] + batches[-1][1]
        batches.append((s, min(SC, CH - s)))
    bi = 0
    batch_end = 0
    g_cur = None
    cur_start = 0
    oh_cur = None
    oh_start = 0
    oh_end = 0
    oi = 0
    P4 = None
    q = 0
    jov = 0
    NB = len(blocks_nov)
    for b, nov in enumerate(blocks_nov):
        if b % 4 == 0:
            P4 = pp.tile([128, 4, 16, PACK], F32, tag="pblk")
        nch = T_ID + nov
        for k in range(nch):
            if q == batch_end:
                cur_start, w = batches[bi]
                g_cur = pool.tile([128, SC, 16, PACK], gdt, tag="gsc")
                eng = nc.sync if bi % 2 == 0 else nc.scalar
                eng.dma_start(out=g_cur[:, :w], in_=g[:, cur_start : cur_start + w])
                batch_end = cur_start + w
                bi += 1
                if pre_hook is not None:
                    pre_hook()
                    pre_hook = None
            if k >= T_ID:
                if jov == oh_end:
                    oh_start = jov
                    wov = min(4 if oi == 0 else SCOV, CHOV - jov)
                    oh_cur = pool.tile([128, SCOV, 128], gdt, tag="ohb")
                    eng = nc.scalar if oi % 2 == 0 else nc.sync
                    eng.dma_start(
                        out=oh_cur[:, :wov], in_=oh[:, oh_start : oh_start + wov]
                    )
                    oh_end = oh_start + wov
                    oi += 1
                lhsT = oh_cur[:, jov - oh_start, :]
                jov += 1
            else:
                lhsT = id_sb[:]
            nc.tensor.matmul(
                out=P4[:, b % 4],
                lhsT=lhsT,
                rhs=g_cur[:, q - cur_start],
                start=(k == 0),
                stop=(k == nch - 1),
            )
            q += 1
        if b % 4 == 3 or b == NB - 1:
            b0 = (b // 4) * 4
            nb = b - b0 + 1
            s4 = pool.tile([128, 4, 16], F32, tag="s4")
            nc.vector.tensor_reduce(
                out=s4[:, :nb],
                in_=P4[:, :nb],
                axis=mybir.AxisListType.X,
                op=ALU.add,
            )
            on_stripe(b0, nb, s4)


def build_B(CH, CHOV, blocks_nov, gdt=BF16):
    """s1 -> agg1 -> relu -> v2 -> t2 = v2 @ W2 (sole output), striped."""
    nc = bass.Bass()
    g = nc.dram_tensor("g", [128, CH, 16, PACK], gdt, kind="ExternalInput")
    oh = nc.dram_tensor("oh", [128, CHOV, 128], gdt, kind="ExternalInput")
    u1own = nc.dram_tensor("u1own", [128, NT, 16], BF16, kind="ExternalInput")
    dinva = nc.dram_tensor("dinva", [128, NT], F32, kind="ExternalInput")
    W2q = nc.dram_tensor("W2q", [64, 4, 16], BF16, kind="ExternalInput")
    identT = nc.dram_tensor("identT", [128, 128], BF16, kind="ExternalInput")
    t2 = nc.dram_tensor("t2", [128, NT, 16], BF16, kind="ExternalOutput")
    with tile.TileContext(nc) as tc:
        with (
            tc.tile_pool(name="sbuf", bufs=3) as pool,
            tc.tile_pool(name="stat", bufs=1) as spool,
            tc.tile_pool(name="psum", bufs=6, space="PSUM") as pp,
            tc.tile_pool(name="psumt", bufs=1, space="PSUM") as ppt,
        ):
            id_sb = spool.tile([128, 128], BF16)
            nc.sync.dma_start(out=id_sb[:], in_=identT[:])
            u1o_bf = spool.tile([128, NT, 16], BF16)
            da = spool.tile([128, NT], F32)
            w2q_sb = spool.tile([64, 4, 16], BF16)
            u1o = spool.tile([128, NT, 16], F32)
            t2_sb = spool.tile([128, NT, 16], BF16)

            def pre_hook():
                nc.scalar.dma_start(out=u1o_bf[:], in_=u1own[:])
                nc.scalar.dma_start(out=da[:], in_=dinva[:])
                nc.scalar.dma_start(out=w2q_sb[:], in_=W2q[:])
                nc.scalar.copy(out=u1o[:], in_=u1o_bf[:])

            def on_stripe(b0, nb, s4):
                sl = slice(b0, b0 + nb)
                agg = pool.tile([128, 4, 16], F32, tag="agg")
                nc.gpsimd.tensor_tensor(
                    out=agg[:, :nb], in0=s4[:, :nb], in1=u1o[:, sl], op=ALU.add
                )
                nc.gpsimd.tensor_tensor(
                    out=agg[:, :nb], in0=agg[:, :nb],
                    in1=da[:, sl].to_broadcast([128, nb, 16]), op=ALU.mult,
                )
                r4 = pool.tile([128, 4, 16], F32, tag="r4")
                nc.scalar.activation(out=r4[:, :nb], in_=agg[:, :nb], func=AF.Relu)
                v4 = pool.tile([128, 4, 16], BF16, tag="v4")
                nc.vector.tensor_tensor(
                    out=v4[:, :nb], in0=r4[:, :nb],
                    in1=da[:, sl].to_broadcast([128, nb, 16]), op=ALU.mult,
                )
                if nb < 4:
                    nc.vector.memset(v4[:, nb:, :], 0.0)
                trps = ppt.tile([64, 128], BF16, tag="trps")
                nc.tensor.transpose(out=trps[:], in_=v4[:], identity=id_sb[:])
                v2T = pool.tile([64, 128], BF16, tag="v2T")
                nc.scalar.copy(out=v2T[:], in_=trps[:])
                z4 = ppt.tile([128, 4, 16], F32, tag="z4")
                for j in range(nb):
                    nc.tensor.matmul(
                        out=z4[:, j], lhsT=v2T[:], rhs=w2q_sb[:, j, :],
                        start=True, stop=True,
                    )
                nc.scalar.copy(out=t2_sb[:, sl, :], in_=z4[:, :nb])
                nc.sync.dma_start(out=t2[:, sl, :], in_=t2_sb[:, sl, :])

            _emit_segsum(
                nc, pool, pp, g, oh, id_sb, blocks_nov, CH, CHOV,
                on_stripe, pre_hook=pre_hook, gdt=gdt,
            )
    return _legalize_waits(nc)


def build_C(CH, CHOV, blocks_nov, gdt=BF16):
    """s2 -> z = dinv*(s2 + t2own) + b2 -> log_softmax, striped."""
    nc = bass.Bass()
    g = nc.dram_tensor("g", [128, CH, 16, PACK], gdt, kind="ExternalInput")
    oh = nc.dram_tensor("oh", [128, CHOV, 128], gdt, kind="ExternalInput")
    t2own = nc.dram_tensor("t2own", [128, NT, 16], BF16, kind="ExternalInput")
    dinva = nc.dram_tensor("dinva", [128, NT], F32, kind="ExternalInput")
    identT = nc.dram_tensor("identT", [128, 128], BF16, kind="ExternalInput")
    outd = nc.dram_tensor("outd", [128, NT, 16], F32, kind="ExternalOutput")
    with tile.TileContext(nc) as tc:
        with (
            tc.tile_pool(name="sbuf", bufs=3) as pool,
            tc.tile_pool(name="stat", bufs=1) as spool,
            tc.tile_pool(name="psum", bufs=8, space="PSUM") as pp,
        ):
            id_sb = spool.tile([128, 128], BF16)
            nc.sync.dma_start(out=id_sb[:], in_=identT[:])
            t2o_bf = spool.tile([128, NT, 16], BF16)
            da = spool.tile([128, NT], F32)
            t2o = spool.tile([128, NT, 16], F32)
            o_sb = spool.tile([128, NT, 16], F32)

            def pre_hook():
                nc.scalar.dma_start(out=t2o_bf[:], in_=t2own[:])
                nc.scalar.dma_start(out=da[:], in_=dinva[:])
                nc.scalar.copy(out=t2o[:], in_=t2o_bf[:])

            def on_stripe(b0, nb, s4):
                sl = slice(b0, b0 + nb)
                z = pool.tile([128, 4, 16], F32, tag="zs")
                nc.gpsimd.tensor_tensor(
                    out=z[:, :nb], in0=s4[:, :nb], in1=t2o[:, sl], op=ALU.add
                )
                nc.gpsimd.tensor_tensor(
                    out=z[:, :nb], in0=z[:, :nb],
                    in1=da[:, sl].to_broadcast([128, nb, 16]), op=ALU.mult,
                )
                m4 = pool.tile([128, 4], F32, tag="m4")
                nc.vector.tensor_reduce(
                    out=m4[:, :nb], in_=z[:, :nb], axis=mybir.AxisListType.X,
                    op=ALU.max,
                )
                zc = pool.tile([128, 4, 16], F32, tag="zc")
                nc.vector.tensor_tensor(
                    out=zc[:, :nb], in0=z[:, :nb],
                    in1=m4[:, :nb].to_broadcast([128, nb, 16]), op=ALU.subtract,
                )
                e4 = pool.tile([128, 4, 16], F32, tag="e4")
                nc.scalar.activation(out=e4[:, :nb], in_=zc[:, :nb], func=AF.Exp)
                ss = pool.tile([128, 4], F32, tag="ss")
                nc.vector.tensor_reduce(
                    out=ss[:, :nb], in_=e4[:, :nb], axis=mybir.AxisListType.X,
                    op=ALU.add,
                )
                lse = pool.tile([128, 4], F32, tag="lse")
                nc.scalar.activation(out=lse[:, :nb], in_=ss[:, :nb], func=AF.Ln)
                nc.vector.tensor_tensor(
                    out=o_sb[:, sl, :], in0=zc[:, :nb],
                    in1=lse[:, :nb].to_broadcast([128, nb, 16]), op=ALU.subtract,
                )
                nc.sync.dma_start(out=outd[:, sl, :], in_=o_sb[:, sl, :])

            _emit_segsum(
                nc, pool, pp, g, oh, id_sb, blocks_nov, CH, CHOV,
                on_stripe, pre_hook=pre_hook, gdt=gdt,
            )
    return _legalize_waits(nc)


# ---------------------------------------------------------------------------
# host side
# ---------------------------------------------------------------------------


def _preprocess(edge_index, n_nodes):
    """Sort edges by dst; build the common chunk structure (T_ID identity +
    n_ov overflow chunks per 128-dst block) + per-core slot metadata."""
    src = np.asarray(edge_index[0])
    dst = np.asarray(edge_index[1])
    deg = np.bincount(dst, minlength=n_nodes).astype(np.float32) + 1.0
    dinv = (1.0 / np.sqrt(deg)).astype(np.float32)

    order = np.argsort(dst, kind="stable")
    sdst = dst[order]
    ssrc = src[order]
    bounds = np.searchsorted(sdst, np.arange(N_CORES + 1) * PER_CORE)

    # per-core local in-degree and slot counts
    deg_loc = np.zeros((N_CORES, PADDED), np.int64)
    core_edges = []
    for c in range(N_CORES):
        lo, hi = bounds[c], bounds[c + 1]
        ld = sdst[lo:hi] - c * PER_CORE
        deg_loc[c, : PER_CORE] = np.bincount(ld, minlength=PER_CORE)
        core_edges.append((ld, ssrc[lo:hi]))
    nslots = -(-deg_loc // PACK)                 # [8, PADDED] ceil div
    ovslots = np.maximum(nslots - T_ID, 0)       # [8, PADDED]

    # common structure: overflow chunk count per block = max over cores
    ov_per_block = ovslots.reshape(N_CORES, NT, 128).sum(axis=2)  # [8, NT]
    n_ov = -(-ov_per_block.max(axis=0) // 128)   # [NT]
    blocks_nov = tuple(int(v) for v in n_ov)
    chunk_base = np.concatenate([[0], np.cumsum(T_ID + n_ov)])    # [NT+1]
    CH = int(chunk_base[-1])
    ov_idx_base = np.concatenate([[0], np.cumsum(n_ov)])          # [NT+1]
    CHOV = max(int(ov_idx_base[-1]), 1)

    sent = N_CORES * PADDED  # sentinel row (zeros) in gather tables
    oh_arrs, sidx_arrs = [], []
    blk_of_dst = np.arange(PADDED) >> 7
    for c in range(N_CORES):
        ov = ovslots[c]
        # exclusive cumsum of overflow slots within each block
        ovc = np.cumsum(ov) - ov
        blk_start = blk_of_dst << 7
        ovbase = ovc - ovc[blk_start]            # [PADDED]
        ld, esrc = core_edges[c]
        gstart = np.concatenate([[0], np.cumsum(deg_loc[c])])
        rank = np.arange(len(ld)) - gstart[ld]
        k_e = rank // PACK
        c_e = rank % PACK
        blk = ld >> 7
        is_id = k_e < T_ID
        q_id = chunk_base[blk] + k_e
        p_id = ld & 127
        ovpos = ovbase[ld] + (k_e - T_ID)
        q_ov = chunk_base[blk] + T_ID + ovpos // 128
        p_ov = ovpos % 128
        q_e = np.where(is_id, q_id, q_ov)
        p_e = np.where(is_id, p_id, p_ov)
        # gather row index: src node -> (core, p, t) -> core*PADDED + p*NT + t
        sc_, rr = esrc // PER_CORE, esrc % PER_CORE
        grow = sc_ * PADDED + (rr % 128) * NT + rr // 128
        sidx = np.full((128, CH, PACK), sent, np.int64)
        sidx[p_e, q_e, c_e] = grow
        # precomputed overflow one-hots [128 slot, CHOV, 128 row]
        oh = np.zeros((128, CHOV, 128), np.uint8)
        m = (~is_id) & (c_e == 0)
        qovc = ov_idx_base[blk[m]] + ovpos[m] // 128
        oh[p_ov[m], qovc, ld[m] & 127] = 1
        oh_arrs.append(oh)
        sidx_arrs.append(sidx)
    return dinv, CH, CHOV, blocks_nov, oh_arrs, sidx_arrs


_CACHE = {}
LAST_HW_NS = None
LAST_TIMES = {}


def _record(tag, res, t_wall):
    global LAST_HW_NS
    LAST_TIMES[tag] = t_wall
    if res.exec_time_ns is not None:
        LAST_HW_NS = (LAST_HW_NS or 0) + res.exec_time_ns


def _gather_g(table, sidx):
    """table [8*PADDED+1, 16] bf16, sidx [128, CH, PACK] -> [128, CH, 16, PACK]."""
    vals = table[sidx]  # [128, CH, PACK, 16]
    return np.ascontiguousarray(vals.transpose(0, 1, 3, 2))


def kernel(x, W1, b1, W2, b2, edge_index):
    global LAST_HW_NS
    LAST_HW_NS = None
    LAST_TIMES.clear()
    import time as _time

    x = np.asarray(x, dtype=np.float32)
    W1 = np.asarray(W1, dtype=np.float32)
    b1 = np.asarray(b1, dtype=np.float32)
    W2 = np.asarray(W2, dtype=np.float32)
    b2 = np.asarray(b2, dtype=np.float32)
    edge_index = np.asarray(edge_index)
    n_nodes, fin = x.shape
    FC = fin // 128

    t0 = _time.time()
    dinv, CH, CHOV, blocks_nov, oh_arrs, sidx_arrs = _preprocess(
        edge_index, n_nodes
    )
    LAST_TIMES["preprocess"] = _time.time() - t0

    key = (n_nodes, CH, CHOV, blocks_nov, G1_FP8, G2_FP8, X_FP8)
    if key not in _CACHE:
        F8 = mybir.dt.float8e4
        _CACHE[key] = (
            build_A(FC, xdt=F8 if X_FP8 else BF16),
            build_B(CH, CHOV, blocks_nov, gdt=F8 if G1_FP8 else BF16),
            build_C(CH, CHOV, blocks_nov, gdt=F8 if G2_FP8 else BF16),
        )
    ncA, ncB, ncC = _CACHE[key]
    cores = list(range(N_CORES))

    # ---- static per-core arrays ----
    t0 = _time.time()
    W1r = np.ascontiguousarray(
        W1.astype(NPBF16).reshape(FC, 128, 16).transpose(1, 0, 2)
    )
    dinva_c = []
    for c in cores:
        dv = np.ones(PADDED, np.float32)
        dv[:PER_CORE] = dinv[c * PER_CORE : (c + 1) * PER_CORE]
        dinva_c.append(np.ascontiguousarray(dv.reshape(NT, 128).T))
    oh1_c = [a.astype(NPF8 if G1_FP8 else NPBF16) for a in oh_arrs]
    oh2_c = (
        oh1_c if G1_FP8 == G2_FP8
        else [a.astype(NPF8 if G2_FP8 else NPBF16) for a in oh_arrs]
    )
    W2bf = W2.astype(NPBF16)
    rdeg_c = []  # sqrt(deg) per core in [128, NT] layout (1/dinva)
    for c in cores:
        rdeg_c.append((1.0 / dinva_c[c]).astype(np.float32))
    w2q = np.zeros((64, 4, 16), NPBF16)
    for j in range(4):
        w2q[16 * j : 16 * j + 16, j] = W2bf
    ident_np = np.eye(128, dtype=np.float32).astype(NPBF16)

    # ---- dispatch A ----
    in_A = []
    xnp = NPF8 if X_FP8 else NPBF16
    for c in cores:
        xs = x[c * PER_CORE : (c + 1) * PER_CORE]
        xp = np.zeros((PADDED, fin), xnp)
        xp[: xs.shape[0]] = xs.astype(xnp)
        xTr = np.ascontiguousarray(
            xp.reshape(NT, 128, FC, 128).transpose(3, 0, 2, 1)
        )  # [128 f_lo, NT, FC, 128 n]
        in_A.append({"xT": xTr, "W1b": W1r, "dinva": dinva_c[c]})
    LAST_TIMES["prepA"] = _time.time() - t0
    t0 = _time.time()
    resA = run_bass_kernel_spmd(ncA, in_A, core_ids=cores)
    _record("dispatchA", resA, _time.time() - t0)
    u1s = [resA.results[c]["u1"] for c in cores]  # [128, NT, 16] bf16

    # ---- host gather for layer 1 ----
    t0 = _time.time()
    table1 = np.concatenate(
        [u1s[c].reshape(PADDED, 16) for c in cores] + [np.zeros((1, 16), NPBF16)],
        axis=0,
    )
    if G1_FP8:
        table1 = table1.astype(NPF8)
    in_B = []
    for c in cores:
        # fold the post-norm bias: dinv*(s + u1own + b1*sqrt(deg)) == dinv*(s+u1own) + b1
        u1f = u1s[c].astype(np.float32) + b1[None, None, :] * rdeg_c[c][:, :, None]
        in_B.append(
            {
                "g": _gather_g(table1, sidx_arrs[c]),
                "oh": oh1_c[c],
                "u1own": u1f.astype(NPBF16),
                "dinva": dinva_c[c],
                "W2q": w2q,
                "identT": ident_np,
            }
        )
    LAST_TIMES["gather1"] = _time.time() - t0
    t0 = _time.time()
    resB = run_bass_kernel_spmd(ncB, in_B, core_ids=cores)
    _record("dispatchB", resB, _time.time() - t0)
    t2s = [resB.results[c]["t2"] for c in cores]

    # ---- host gather for layer 2 ----
    t0 = _time.time()
    table2 = np.concatenate(
        [t2s[c].reshape(PADDED, 16) for c in cores] + [np.zeros((1, 16), NPBF16)],
        axis=0,
    )
    if G2_FP8:
        table2 = table2.astype(NPF8)
    in_C = []
    for c in cores:
        t2f = t2s[c].astype(np.float32) + b2[None, None, :] * rdeg_c[c][:, :, None]
        in_C.append(
            {
                "g": _gather_g(table2, sidx_arrs[c]),
                "oh": oh2_c[c],
                "t2own": t2f.astype(NPBF16),
                "dinva": dinva_c[c],
                "identT": ident_np,
            }
        )
    LAST_TIMES["gather2"] = _time.time() - t0
    t0 = _time.time()
    resC = run_bass_kernel_spmd(ncC, in_C, core_ids=cores)
    _record("dispatchC", resC, _time.time() - t0)
    out = np.concatenate(
        [
            resC.results[c]["outd"].transpose(1, 0, 2).reshape(PADDED, 16)[:PER_CORE]
            for c in cores
        ],
        axis=0,
    ).astype(np.float32)
    return out


# revision 47
# speedup vs baseline: 1.0454x; 1.0149x over previous
"""GCN (2-layer, PyG GCNConv semantics) on 8 Trainium2 NeuronCores.

Strategy (dst-shard, graph-parallel), v2:
- Nodes sharded contiguously across 8 cores (12500 dsts/core).
- 3 SPMD dispatches:
    A: u1 = dinv * (x @ W1)            (x pre-transposed bf16, 4KB DMA runs)
    B: s1 = segsum(g1); agg1 = dinv*(s1+u1own)+b1; r1 = relu;
       v2 = dinv*r1; t2 = v2 @ W2      (outputs only t2, 0.4MB)
    C: s2 = segsum(g2); z = dinv*(s2+t2own)+b2; out = log_softmax(z)
- Segment-sum: edges packed 8-per-slot by destination; per 128-dst
  block, the first T_ID=4 slots of every dst go to "identity" chunks
  (slot partition == dst row, lhsT = static identity - no one-hot
  work), remaining slots to ~1 "overflow" chunk routed by an is_equal
  one-hot. All chunks of a block accumulate into one PSUM tile
  [128, 16f, 8sub]; one DVE reduce per block sums the 8 subslots.
  Chunk structure is common across cores (max-over-cores sizing) so a
  single SPMD program serves all 8 cores.
- The two per-edge value gathers (u1[src]/t2[src] for 3.2M edges) run
  on the host between dispatches (every on-device gather primitive in
  this toolchain was measured unusable: indirect DMA ~1.6us/row,
  GPSIMD gather ucode unloadable under this walrus build).
"""
import os
import sys
import numpy as np

sys.path.insert(0, "/opt/trn_rl_repo")

try:
    # NTFF profiling glue: the image's antenv lacks axon_hooks, which makes
    # run_bass_kernel_spmd(trace=True) crash. Provide it (and register the
    # ctypes hook) so tracing works when BASS_TRACE is set; harmless if not.
    import types as _types

    if "antenv.axon_hooks" not in sys.modules:
        _m = _types.ModuleType("antenv.axon_hooks")
        _st = {}
        _m.set_axon_ntff_profile_hook = lambda h: _st.__setitem__("h", h)
        _m.get_axon_ntff_profile_hook = lambda: _st.get("h")
        sys.modules["antenv.axon_hooks"] = _m
        from trn_agent_boot.trn_boot import _ntff_profile_via_ctypes

        _m.set_axon_ntff_profile_hook(
            _ntff_profile_via_ctypes("/opt/axon/libaxon_pjrt.so")
        )
except Exception:
    pass

import ml_dtypes
import concourse.bass as bass
import concourse.mybir as mybir
import concourse.tile as tile
from concourse.vector_clock import ScopedClock
import concourse.bass_utils as _bu
from concourse.bass_utils import run_bass_kernel_spmd

_orig_upload = _bu.upload_artifacts


def _safe_upload(tmpdir):
    try:
        return _orig_upload(tmpdir)
    except Exception:
        return "local://" + tmpdir


_bu.upload_artifacts = _safe_upload

BF16 = mybir.dt.bfloat16
F32 = mybir.dt.float32
AF = mybir.ActivationFunctionType
ALU = mybir.AluOpType
NPBF16 = ml_dtypes.bfloat16
NPF8 = ml_dtypes.float8_e4m3

G1_FP8 = True    # layer-1 gathered values in fp8 (e4m3)
G2_FP8 = True    # layer-2 gathered values dtype
X_FP8 = True     # x (dispatch A input) dtype

N_CORES = 8
PER_CORE = 12500
NT = 98              # 128-dst tiles per core (12544 padded)
PADDED = NT * 128
PACK = 8             # edges per slot (matmul N = 16 feats x PACK)
T_ID = 4             # identity chunks per block (slots 0..3 of each dst)
SC = 32              # chunks per g superchunk (DMA batch)
SCOV = 16            # overflow chunks per is_equal batch

# ---------------------------------------------------------------------------
# walrus workaround: only ONE sync-wait command per instruction is accepted.
# ---------------------------------------------------------------------------


def _patched_drain_and_barrier(self, tick_clock, wait_clock):
    nc = self.nc
    carrier = nc.sync.nop(nofuse=True, hint="drain_wait_carrier")
    wait_clock.add_sem_waits(carrier.ins, ScopedClock({None: tick_clock.global_clock}))
    si = carrier.ins.sync_info
    waits = list(si.on_wait or []) if si else []
    if len(waits) > 1:
        si.on_wait = waits[:1]
        for i in range(1, len(waits)):
            extra = nc.sync.nop(nofuse=True, hint="drain_wait_carrier")
            extra.ins.sync_info = mybir.SyncInfo(on_wait=waits[i : i + 1], on_update=[])
    nc.sync.drain()
    nc.all_engine_barrier()
    assert self.sems is not None
    popped = nc._tile_sem_poison_stack.pop()
    assert popped is self._sem_poison
    nc.clear_and_free_semaphores(list(self.sems.allocated().values()))
    nc.all_engine_barrier()


tile.TileContext._drain_and_barrier = _patched_drain_and_barrier


def _legalize_waits(nc, max_waits=1):
    n = [0]

    def mk_nop(engine, waits):
        n[0] += 1
        return mybir.InstNoOp(
            name=f"waitnop-{n[0]}",
            engine=engine,
            ins=[],
            outs=[],
            sync_info=mybir.SyncInfo(on_wait=list(waits), on_update=[]),
            text_hint="wait_carrier",
        )

    for f in nc.m.functions:
        for bb in f.blocks:
            out = []
            changed = False
            for inst in bb.instructions:
                si = inst.sync_info
                waits = list(si.on_wait or []) if si else []
                if len(waits) > max_waits:
                    changed = True
                    for i in range(0, len(waits) - max_waits, max_waits):
                        out.append(mk_nop(inst.engine, waits[i : i + max_waits]))
                    si.on_wait = waits[len(waits) - max_waits :]
                out.append(inst)
            if changed:
                bb.instructions = out
    return nc


# ---------------------------------------------------------------------------
# device kernel builders
# ---------------------------------------------------------------------------


def build_A(FC=4, xdt=BF16):
    """u1 = dinv * (x @ W1). xT host layout [128, NT, FC, 128]."""
    nc = bass.Bass()
    xT = nc.dram_tensor("xT", [128, NT, FC, 128], xdt, kind="ExternalInput")
    W1b = nc.dram_tensor("W1b", [128, FC, 16], BF16, kind="ExternalInput")
    dinva = nc.dram_tensor("dinva", [128, NT], F32, kind="ExternalInput")
    u1 = nc.dram_tensor("u1", [128, NT, 16], BF16, kind="ExternalOutput")
    TB = 8  # node-tiles per DMA batch (8KB per partition)
    with tile.TileContext(nc) as tc:
        with (
            tc.tile_pool(name="sbuf", bufs=3) as pool,
            tc.tile_pool(name="stat", bufs=1) as spool,
            tc.tile_pool(name="psum", bufs=8, space="PSUM") as pp,
        ):
            w1 = spool.tile([128, FC, 16], BF16)
            nc.sync.dma_start(out=w1[:], in_=W1b[:])
            da = spool.tile([128, NT], F32)
            nc.scalar.dma_start(out=da[:], in_=dinva[:])
            u1_sb = spool.tile([128, NT, 16], BF16)
            batches = [(0, 2), (2, 6)]
            while batches[-1][0] + batches[-1][1] < NT:
                s = batches[-1][0] + batches[-1][1]
                batches.append((s, min(TB, NT - s)))
            for bi, (t0, tb) in enumerate(batches):
                xt = pool.tile([128, TB, FC, 128], xdt, tag="xt")
                eng = (nc.sync, nc.scalar, nc.gpsimd)[bi % 3]
                eng.dma_start(out=xt[:, :tb], in_=xT[:, t0 : t0 + tb])
                for i in range(tb):
                    ps = pp.tile([128, 16], F32, tag="hps")
                    for fc in range(FC):
                        nc.tensor.matmul(
                            out=ps[:],
                            lhsT=xt[:, i, fc, :],
                            rhs=w1[:, fc, :],
                            start=(fc == 0),
                            stop=(fc == FC - 1),
                        )
                    t = t0 + i
                    nc.vector.tensor_tensor(
                        out=u1_sb[:, t, :],
                        in0=ps[:],
                        in1=da[:, t : t + 1].to_broadcast([128, 16]),
                        op=ALU.mult,
                    )
            nc.sync.dma_start(out=u1[:], in_=u1_sb[:])
    return _legalize_waits(nc)


def _emit_segsum(
    nc, pool, pp, g, oh, id_sb, blocks_nov, CH, CHOV, on_stripe,
    pre_hook=None, gdt=BF16,
):
    """Per-block psum scatter + subslot reduce, delivered in 4-block stripes.

    Per block: T_ID identity chunks (lhsT = id_sb) + blocks_nov[b] overflow
    chunks (lhsT = host-precomputed one-hot slices streamed from `oh`). All
    chunks of a block accumulate into one PSUM sub-tile; 4 blocks share a
    bank. After each stripe's DVE reduce, on_stripe(b0, nb, s4) consumes the
    [128, nb, 16] f32 stripe so the epilogue overlaps the remaining scatter.

    g and oh DMAs use staged schedules (small first batches so the PE starts
    fast); pre_hook() is emitted right after the first g DMA so secondary
    input loads queue behind it."""
    batches = [(0, 8), (8, 24)]
    while batches[-1][0] + batches[-1][1] < CH:
        s = batches[-1][0] + batches[-1][1]
        batches.append((s, min(SC, CH - s)))
    bi = 0
    batch_end = 0
    g_cur = None
    cur_start = 0
    oh_cur = None
    oh_start = 0
    oh_end = 0
    oi = 0
    P4 = None
    q = 0
    jov = 0
    NB = len(blocks_nov)
    for b, nov in enumerate(blocks_nov):
        if b % 4 == 0:
            P4 = pp.tile([128, 4, 16, PACK], F32, tag="pblk")
        nch = T_ID + nov
        for k in range(nch):
            if q == batch_end:
                cur_start, w = batches[bi]
                g_cur = pool.tile([128, SC, 16, PACK], gdt, tag="gsc")
                eng = nc.sync if bi % 2 == 0 else nc.scalar
                eng.dma_start(out=g_cur[:, :w], in_=g[:, cur_start : cur_start + w])
                batch_end = cur_start + w
                bi += 1
                if pre_hook is not None:
                    pre_hook()
                    pre_hook = None
            if k >= T_ID:
                if jov == oh_end:
                    oh_start = jov
                    wov = min(4 if oi == 0 else SCOV, CHOV - jov)
                    oh_cur = pool.tile([128, SCOV, 128], gdt, tag="ohb")
                    eng = nc.scalar if oi % 2 == 0 else nc.sync
                    eng.dma_start(
                        out=oh_cur[:, :wov], in_=oh[:, oh_start : oh_start + wov]
                    )
                    oh_end = oh_start + wov
                    oi += 1
                lhsT = oh_cur[:, jov - oh_start, :]
                jov += 1
            else:
                lhsT = id_sb[:]
            nc.tensor.matmul(
                out=P4[:, b % 4],
                lhsT=lhsT,
                rhs=g_cur[:, q - cur_start],
                start=(k == 0),
                stop=(k == nch - 1),
            )
            q += 1
        if b % 4 == 3 or b == NB - 1:
            b0 = (b // 4) * 4
            nb = b - b0 + 1
            s4 = pool.tile([128, 4, 16], F32, tag="s4")
            nc.vector.tensor_reduce(
                out=s4[:, :nb],
                in_=P4[:, :nb],
                axis=mybir.AxisListType.X,
                op=ALU.add,
            )
            on_stripe(b0, nb, s4)


def build_B(CH, CHOV, blocks_nov, gdt=BF16):
    """s1 -> agg1 -> relu -> v2 -> t2 = v2 @ W2 (sole output), striped."""
    nc = bass.Bass()
    g = nc.dram_tensor("g", [128, CH, 16, PACK], gdt, kind="ExternalInput")
    oh = nc.dram_tensor("oh", [128, CHOV, 128], gdt, kind="ExternalInput")
    u1own = nc.dram_tensor("u1own", [128, NT, 16], BF16, kind="ExternalInput")
    dinva = nc.dram_tensor("dinva", [128, NT], F32, kind="ExternalInput")
    W2q = nc.dram_tensor("W2q", [64, 4, 16], BF16, kind="ExternalInput")
    identT = nc.dram_tensor("identT", [128, 128], BF16, kind="ExternalInput")
    t2 = nc.dram_tensor("t2", [128, NT, 16], BF16, kind="ExternalOutput")
    with tile.TileContext(nc) as tc:
        with (
            tc.tile_pool(name="sbuf", bufs=3) as pool,
            tc.tile_pool(name="stat", bufs=1) as spool,
            tc.tile_pool(name="psum", bufs=6, space="PSUM") as pp,
            tc.tile_pool(name="psumt", bufs=1, space="PSUM") as ppt,
        ):
            id_sb = spool.tile([128, 128], BF16)
            nc.sync.dma_start(out=id_sb[:], in_=identT[:])
            u1o_bf = spool.tile([128, NT, 16], BF16)
            da = spool.tile([128, NT], F32)
            w2q_sb = spool.tile([64, 4, 16], BF16)
            u1o = spool.tile([128, NT, 16], F32)
            t2_sb = spool.tile([128, NT, 16], BF16)

            def pre_hook():
                nc.scalar.dma_start(out=u1o_bf[:], in_=u1own[:])
                nc.scalar.dma_start(out=da[:], in_=dinva[:])
                nc.scalar.dma_start(out=w2q_sb[:], in_=W2q[:])
                nc.scalar.copy(out=u1o[:], in_=u1o_bf[:])

            def on_stripe(b0, nb, s4):
                sl = slice(b0, b0 + nb)
                agg = pool.tile([128, 4, 16], F32, tag="agg")
                nc.gpsimd.tensor_tensor(
                    out=agg[:, :nb], in0=s4[:, :nb], in1=u1o[:, sl], op=ALU.add
                )
                nc.gpsimd.tensor_tensor(
                    out=agg[:, :nb], in0=agg[:, :nb],
                    in1=da[:, sl].to_broadcast([128, nb, 16]), op=ALU.mult,
                )
                r4 = pool.tile([128, 4, 16], F32, tag="r4")
                nc.scalar.activation(out=r4[:, :nb], in_=agg[:, :nb], func=AF.Relu)
                v4 = pool.tile([128, 4, 16], BF16, tag="v4")
                nc.vector.tensor_tensor(
                    out=v4[:, :nb], in0=r4[:, :nb],
                    in1=da[:, sl].to_broadcast([128, nb, 16]), op=ALU.mult,
                )
                if nb < 4:
                    nc.vector.memset(v4[:, nb:, :], 0.0)
                trps = ppt.tile([64, 128], BF16, tag="trps")
                nc.tensor.transpose(out=trps[:], in_=v4[:], identity=id_sb[:])
                v2T = pool.tile([64, 128], BF16, tag="v2T")
                nc.scalar.copy(out=v2T[:], in_=trps[:])
                z4 = ppt.tile([128, 4, 16], F32, tag="z4")
                for j in range(nb):
                    nc.tensor.matmul(
                        out=z4[:, j], lhsT=v2T[:], rhs=w2q_sb[:, j, :],
                        start=True, stop=True,
                    )
                nc.scalar.copy(out=t2_sb[:, sl, :], in_=z4[:, :nb])
                nc.sync.dma_start(out=t2[:, sl, :], in_=t2_sb[:, sl, :])

            _emit_segsum(
                nc, pool, pp, g, oh, id_sb, blocks_nov, CH, CHOV,
                on_stripe, pre_hook=pre_hook, gdt=gdt,
            )
    return _legalize_waits(nc)


def build_C(CH, CHOV, blocks_nov, gdt=BF16):
    """s2 -> z = dinv*(s2 + t2own) + b2 -> log_softmax, striped."""
    nc = bass.Bass()
    g = nc.dram_tensor("g", [128, CH, 16, PACK], gdt, kind="ExternalInput")
    oh = nc.dram_tensor("oh", [128, CHOV, 128], gdt, kind="ExternalInput")
    t2own = nc.dram_tensor("t2own", [128, NT, 16], BF16, kind="ExternalInput")
    dinva = nc.dram_tensor("dinva", [128, NT], F32, kind="ExternalInput")
    identT = nc.dram_tensor("identT", [128, 128], BF16, kind="ExternalInput")
    outd = nc.dram_tensor("outd", [128, NT, 16], F32, kind="ExternalOutput")
    with tile.TileContext(nc) as tc:
        with (
            tc.tile_pool(name="sbuf", bufs=3) as pool,
            tc.tile_pool(name="stat", bufs=1) as spool,
            tc.tile_pool(name="psum", bufs=8, space="PSUM") as pp,
        ):
            id_sb = spool.tile([128, 128], BF16)
            nc.sync.dma_start(out=id_sb[:], in_=identT[:])
            t2o_bf = spool.tile([128, NT, 16], BF16)
            da = spool.tile([128, NT], F32)
            t2o = spool.tile([128, NT, 16], F32)
            o_sb = spool.tile([128, NT, 16], F32)

            def pre_hook():
                nc.scalar.dma_start(out=t2o_bf[:], in_=t2own[:])
                nc.scalar.dma_start(out=da[:], in_=dinva[:])
                nc.scalar.copy(out=t2o[:], in_=t2o_bf[:])

            def on_stripe(b0, nb, s4):
                sl = slice(b0, b0 + nb)
                z = pool.tile([128, 4, 16], F32, tag="zs")
                nc.gpsimd.tensor_tensor(
                    out=z[:, :nb], in0=s4[:, :nb], in1=t2o[:, sl], op=ALU.add
                )
                nc.gpsimd.tensor_tensor(
                    out=z[:, :nb], in0=z[:, :nb],
                    in1=da[:, sl].to_broadcast([128, nb, 16]), op=ALU.mult,
                )
                m4 = pool.tile([128, 4], F32, tag="m4")
                nc.vector.tensor_reduce(
                    out=m4[:, :nb], in_=z[:, :nb], axis=mybir.AxisListType.X,
                    op=ALU.max,
                )
                zc = pool.tile([128, 4, 16], F32, tag="zc")
                nc.vector.tensor_tensor(
                    out=zc[:, :nb], in0=z[:, :nb],
                    in1=m4[:, :nb].to_broadcast([128, nb, 16]), op=ALU.subtract,
                )
                e4 = pool.tile([128, 4, 16], F32, tag="e4")
                nc.scalar.activation(out=e4[:, :nb], in_=zc[:, :nb], func=AF.Exp)
                ss = pool.tile([128, 4], F32, tag="ss")
                nc.vector.tensor_reduce(
                    out=ss[:, :nb], in_=e4[:, :nb], axis=mybir.AxisListType.X,
                    op=ALU.add,
                )
                lse = pool.tile([128, 4], F32, tag="lse")
                nc.scalar.activation(out=lse[:, :nb], in_=ss[:, :nb], func=AF.Ln)
                nc.vector.tensor_tensor(
                    out=o_sb[:, sl, :], in0=zc[:, :nb],
                    in1=lse[:, :nb].to_broadcast([128, nb, 16]), op=ALU.subtract,
                )
                nc.sync.dma_start(out=outd[:, sl, :], in_=o_sb[:, sl, :])

            _emit_segsum(
                nc, pool, pp, g, oh, id_sb, blocks_nov, CH, CHOV,
                on_stripe, pre_hook=pre_hook, gdt=gdt,
            )
    return _legalize_waits(nc)


# ---------------------------------------------------------------------------
# host side
# ---------------------------------------------------------------------------


def _preprocess(edge_index, n_nodes):
    """Sort edges by dst; build the common chunk structure (T_ID identity +
    n_ov overflow chunks per 128-dst block) + per-core slot metadata."""
    src = np.asarray(edge_index[0])
    dst = np.asarray(edge_index[1])
    deg = np.bincount(dst, minlength=n_nodes).astype(np.float32) + 1.0
    dinv = (1.0 / np.sqrt(deg)).astype(np.float32)

    order = np.argsort(dst, kind="stable")
    sdst = dst[order]
    ssrc = src[order]
    bounds = np.searchsorted(sdst, np.arange(N_CORES + 1) * PER_CORE)

    # per-core local in-degree and slot counts
    deg_loc = np.zeros((N_CORES, PADDED), np.int64)
    core_edges = []
    for c in range(N_CORES):
        lo, hi = bounds[c], bounds[c + 1]
        ld = sdst[lo:hi] - c * PER_CORE
        deg_loc[c, : PER_CORE] = np.bincount(ld, minlength=PER_CORE)
        core_edges.append((ld, ssrc[lo:hi]))
    nslots = -(-deg_loc // PACK)                 # [8, PADDED] ceil div
    ovslots = np.maximum(nslots - T_ID, 0)       # [8, PADDED]

    # common structure: overflow chunk count per block = max over cores
    ov_per_block = ovslots.reshape(N_CORES, NT, 128).sum(axis=2)  # [8, NT]
    n_ov = -(-ov_per_block.max(axis=0) // 128)   # [NT]
    blocks_nov = tuple(int(v) for v in n_ov)
    chunk_base = np.concatenate([[0], np.cumsum(T_ID + n_ov)])    # [NT+1]
    CH = int(chunk_base[-1])
    ov_idx_base = np.concatenate([[0], np.cumsum(n_ov)])          # [NT+1]
    CHOV = max(int(ov_idx_base[-1]), 1)

    sent = N_CORES * PADDED  # sentinel row (zeros) in gather tables
    oh_arrs, sidx_arrs = [], []
    blk_of_dst = np.arange(PADDED) >> 7
    for c in range(N_CORES):
        ov = ovslots[c]
        # exclusive cumsum of overflow slots within each block
        ovc = np.cumsum(ov) - ov
        blk_start = blk_of_dst << 7
        ovbase = ovc - ovc[blk_start]            # [PADDED]
        ld, esrc = core_edges[c]
        gstart = np.concatenate([[0], np.cumsum(deg_loc[c])])
        rank = np.arange(len(ld)) - gstart[ld]
        k_e = rank // PACK
        c_e = rank % PACK
        blk = ld >> 7
        is_id = k_e < T_ID
        q_id = chunk_base[blk] + k_e
        p_id = ld & 127
        ovpos = ovbase[ld] + (k_e - T_ID)
        q_ov = chunk_base[blk] + T_ID + ovpos // 128
        p_ov = ovpos % 128
        q_e = np.where(is_id, q_id, q_ov)
        p_e = np.where(is_id, p_id, p_ov)
        # gather row index: src node -> (core, p, t) -> core*PADDED + p*NT + t
        sc_, rr = esrc // PER_CORE, esrc % PER_CORE
        grow = sc_ * PADDED + (rr % 128) * NT + rr // 128
        sidx = np.full((128, CH, PACK), sent, np.int64)
        sidx[p_e, q_e, c_e] = grow
        # precomputed overflow one-hots [128 slot, CHOV, 128 row]
        oh = np.zeros((128, CHOV, 128), np.uint8)
        m = (~is_id) & (c_e == 0)
        qovc = ov_idx_base[blk[m]] + ovpos[m] // 128
        oh[p_ov[m], qovc, ld[m] & 127] = 1
        oh_arrs.append(oh)
        sidx_arrs.append(sidx)
    return dinv, CH, CHOV, blocks_nov, oh_arrs, sidx_arrs


_CACHE = {}
LAST_HW_NS = None
LAST_TIMES = {}


def _record(tag, res, t_wall):
    global LAST_HW_NS
    LAST_TIMES[tag] = t_wall
    if res.exec_time_ns is not None:
        LAST_HW_NS = (LAST_HW_NS or 0) + res.exec_time_ns


def _gather_g(table, sidx):
    """table [8*PADDED+1, 16] bf16, sidx [128, CH, PACK] -> [128, CH, 16, PACK]."""
    vals = table[sidx]  # [128, CH, PACK, 16]
    return np.ascontiguousarray(vals.transpose(0, 1, 3, 2))


def kernel(x, W1, b1, W2, b2, edge_index):
    global LAST_HW_NS
    LAST_HW_NS = None
    LAST_TIMES.clear()
    import time as _time

    x = np.asarray(x, dtype=np.float32)
    W1 = np.asarray(W1, dtype=np.float32)
    b1 = np.asarray(b1, dtype=np.float32)
    W2 = np.asarray(W2, dtype=np.float32)
    b2 = np.asarray(b2, dtype=np.float32)
    edge_index = np.asarray(edge_index)
    n_nodes, fin = x.shape
    FC = fin // 128

    t0 = _time.time()
    dinv, CH, CHOV, blocks_nov, oh_arrs, sidx_arrs = _preprocess(
        edge_index, n_nodes
    )
    LAST_TIMES["preprocess"] = _time.time() - t0

    key = (n_nodes, CH, CHOV, blocks_nov, G1_FP8, G2_FP8, X_FP8)
    if key not in _CACHE:
        F8 = mybir.dt.float8e4
        _CACHE[key] = (
            build_A(FC, xdt=F8 if X_FP8 else BF16),
            build_B(CH, CHOV, blocks_nov, gdt=F8 if G1_FP8 else BF16),
            build_C(CH, CHOV, blocks_nov, gdt=F8 if G2_FP8 else BF16),
        )
    ncA, ncB, ncC = _CACHE[key]
    cores = list(range(N_CORES))

    # ---- static per-core arrays ----
    t0 = _time.time()
    W1r = np.ascontiguousarray(
        W1.astype(NPBF16).reshape(FC, 128, 16).transpose(1, 0, 2)
    )
    dinva_c = []
    for c in cores:
        dv = np.ones(PADDED, np.float32)
        dv[:PER_CORE] = dinv[c * PER_CORE : (c + 1) * PER_CORE]
        dinva_c.append(np.ascontiguousarray(dv.reshape(NT, 128).T))
    oh1_c = [a.astype(NPF8 if G1_FP8 else NPBF16) for a in oh_arrs]
    oh2_c = (
        oh1_c if G1_FP8 == G2_FP8
        else [a.astype(NPF8 if G2_FP8 else NPBF16) for a in oh_arrs]
    )
    W2bf = W2.astype(NPBF16)
    rdeg_c = []  # sqrt(deg) per core in [128, NT] layout (1/dinva)
    for c in cores:
        rdeg_c.append((1.0 / dinva_c[c]).astype(np.float32))
    w2q = np.zeros((64, 4, 16), NPBF16)
    for j in range(4):
        w2q[16 * j : 16 * j + 16, j] = W2bf
    ident_np = np.eye(128, dtype=np.float32).astype(NPBF16)

    # ---- dispatch A ----
    in_A = []
    xnp = NPF8 if X_FP8 else NPBF16
    for c in cores:
        xs = x[c * PER_CORE : (c + 1) * PER_CORE]
        xp = np.zeros((PADDED, fin), xnp)
        xp[: xs.shape[0]] = xs.astype(xnp)
        xTr = np.ascontiguousarray(
            xp.reshape(NT, 128, FC, 128).transpose(3, 0, 2, 1)
        )  # [128 f_lo, NT, FC, 128 n]
        in_A.append({"xT": xTr, "W1b": W1r, "dinva": dinva_c[c]})
    LAST_TIMES["prepA"] = _time.time() - t0
    t0 = _time.time()
    resA = run_bass_kernel_spmd(ncA, in_A, core_ids=cores)
    _record("dispatchA", resA, _time.time() - t0)
    u1s = [resA.results[c]["u1"] for c in cores]  # [128, NT, 16] bf16

    # ---- host gather for layer 1 ----
    t0 = _time.time()
    table1 = np.concatenate(
        [u1s[c].reshape(PADDED, 16) for c in cores] + [np.zeros((1, 16), NPBF16)],
        axis=0,
    )
    if G1_FP8:
        table1 = table1.astype(NPF8)
    in_B = []
    for c in cores:
        # fold the post-norm bias: dinv*(s + u1own + b1*sqrt(deg)) == dinv*(s+u1own) + b1
        u1f = u1s[c].astype(np.float32) + b1[None, None, :] * rdeg_c[c][:, :, None]
        in_B.append(
            {
                "g": _gather_g(table1, sidx_arrs[c]),
                "oh": oh1_c[c],
                "u1own": u1f.astype(NPBF16),
                "dinva": dinva_c[c],
                "W2q": w2q,
                "identT": ident_np,
            }
        )
    LAST_TIMES["gather1"] = _time.time() - t0
    t0 = _time.time()
    resB = run_bass_kernel_spmd(ncB, in_B, core_ids=cores)
    _record("dispatchB", resB, _time.time() - t0)
    t2s = [resB.results[c]["t2"] for c in cores]

    # ---- host gather for layer 2 ----
    t0 = _time.time()
    table2 = np.concatenate(
        [t2s[c].reshape(PADDED, 16) for c in cores] + [np.zeros((1, 16), NPBF16)],
        axis=0,
    )
    if G2_FP8:
        table2 = table2.astype(NPF8)
    in_C = []
    for c in cores:
        t2f = t2s[c].astype(np.float32) + b2[None, None, :] * rdeg_c[c][:, :, None]
        in_C.append(
            {
                "g": _gather_g(table2, sidx_arrs[c]),
                "oh": oh2_c[c],
                "t2own": t2f.astype(NPBF16),
                "dinva": dinva_c[c],
                "identT": ident_np,
            }
        )
    LAST_TIMES["gather2"] = _time.time() - t0
    t0 = _time.time()
    resC = run_bass_kernel_spmd(ncC, in_C, core_ids=cores)
    _record("dispatchC", resC, _time.time() - t0)
    out = np.concatenate(
        [
            resC.results[c]["outd"].transpose(1, 0, 2).reshape(PADDED, 16)[:PER_CORE]
            for c in cores
        ],
        axis=0,
    ).astype(np.float32)
    return out
